# revision 42
# baseline (speedup 1.0000x reference)
"""Trainium2 Bass kernel for nn_DifferentiablePriorityBuffer (8 NeuronCores), v2.

Same math as v1 (single-round closed form, rel err ~9e-7 vs 10-round scan):

    pooled = (qs * mask).sum(T) / (mask.sum(T) + 1e-8)          (B, D)
    q      = (pooled @ Wq.T + bq)                               (B, D)
    g      = sigmoid((eff - 0.5) * 10) * eff * valid / sqrt(D)
    E      = exp((q @ K.T) * g) * valid                         (B, N)
    out    = (E @ V) / E.sum(-1, keepdims=True)                 (B, D)
    final  = out @ Wc.T + bc                                    (B, D)

v7 (this file) vs v5:
  - Pause-free bulk stream: the AG#1/AG#2-dependent small loads (pooled_full,
    qT) ride the idle SWDGE (gpsimd) queue instead of pausing the HWDGE sync
    FIFO, so the 68MB/core qs+Wq+K+V+Wc stream never yields to a collective.
  - K groups emitted contiguously; the pooledT/qproj/AG#2 block sits between
    K groups 4/5 so its PE work overlaps the K stream without blocking the
    later K transposes for long.
  - Wc streamed AFTER V (wcT is only needed post-AllReduce), taking its
    5.6us off the EV-completion critical path.
  - stream pool 5 bufs -> 10MB of V prefetched across the scores/et wait.
  - Keep-warm matmuls in the pooling phase and across the K-transpose ->
    scores idle window (paced by V-tile arrivals) hold the PE pstate (HAM
    clock gate) up between data-gated bursts.
  - qT / outT gathers merged into single SWDGE DMAs.
  - Last V tile processed nsl-outer: each 512-col E@V slice finalizes as
    soon as possible and its transpose + ar_in store overlaps the other
    slices' matmuls, cutting the EV -> AllReduce serial tail from ~6us
    to ~2us.

v5 scheduling changes vs v1 (v1: 319us NEFF, DMA busy 200us with a 110us
compute/collective tail):
  - All bulk loads (qs, K, V, Wq, Wc) moved to HWDGE (nc.sync) in f32:
    the sync FIFO serializes them in priority order qs > Wq > K > Wc > V
    while streaming at full HBM rate, and the GpSimd queue only carries
    collectives + small transfers, so collective triggers fire as soon
    as their data is ready (v1 lost ~34us to K-load DMAs queued ahead of
    the qT AllGather trigger).
  - float32r matmuls consume the f32 data directly (1 cycle/row at
    >=256 moving cols, vs 4 for plain f32) - no bf16 cast DMAs needed.
    K/W are cast to bf16 on the PSUM->SBUF copies after their PE
    transposes.
  - A dummy warmup AllGather absorbs the one-time ~11us collective
    trigger-start delay while DMA streams.
  - All PSUM->SBUF copies on the vector engine (scalar ACT copies are
    2-9x slower); scalar only runs activations.
  - One shared streaming pool (qs/K/V/W tiles, 5 x 2MB) keeps SBUF
    bounded and lets buffer recycling pace the load FIFO.
"""
import sys

if "/opt/trn_rl_repo" not in sys.path:
    sys.path.insert(0, "/opt/trn_rl_repo")

import math

import numpy as np

import concourse.bacc as bacc
import concourse.tile as tile
from concourse import mybir
from concourse.bass_utils import run_bass_kernel_spmd
from concourse.masks import make_identity

N_CORES = 8
B, T, D = 64, 512, 2048
N = 16384
DECAY = 0.9
THR = 0.5
BL = B // N_CORES          # 8 batches per core
NL = N // N_CORES          # 2048 buffer rows per core
JL = D // N_CORES          # 256 output features per core
DC = D // 128              # 16 contraction chunks
IC = NL // 128             # 16 local buffer chunks
TC = T // 128              # 4 time chunks
F32 = mybir.dt.float32
F32R = mybir.dt.float32r
BF16 = mybir.dt.bfloat16
AF = mybir.ActivationFunctionType

_NC_CACHE = None


def build_nc():
    nc = bacc.Bacc("TRN2", target_bir_lowering=False, debug=False,
                   num_devices=N_CORES)

    qs = nc.dram_tensor("qs", [BL, T, D], F32R, kind="ExternalInput")
    am = nc.dram_tensor("am", [BL, T], F32R, kind="ExternalInput")
    keys = nc.dram_tensor("keys", [NL, D], F32R, kind="ExternalInput")
    values = nc.dram_tensor("values", [NL, D], F32R, kind="ExternalInput")
    pri = nc.dram_tensor("pri", [NL], F32, kind="ExternalInput")
    ages = nc.dram_tensor("ages", [NL], F32, kind="ExternalInput")
    validf = nc.dram_tensor("validf", [NL], F32, kind="ExternalInput")
    wq = nc.dram_tensor("wq", [JL, D], F32R, kind="ExternalInput")
    bq = nc.dram_tensor("bq", [JL], F32, kind="ExternalInput")
    wc = nc.dram_tensor("wc", [JL, D], F32R, kind="ExternalInput")
    bc = nc.dram_tensor("bc", [JL], F32, kind="ExternalInput")
    y = nc.dram_tensor("out", [B, JL], F32, kind="ExternalOutput")

    rg = [list(range(N_CORES))]

    with tile.TileContext(nc) as tc:
        with (
            tc.tile_pool(name="small", bufs=1) as small,
            tc.tile_pool(name="stream", bufs=5) as stream,
            tc.tile_pool(name="persist", bufs=1) as persist,
            tc.tile_pool(name="stage", bufs=2) as stage,
            tc.tile_pool(name="ps_acc", bufs=1, space="PSUM") as ps_acc,
            tc.tile_pool(name="ps_sm", bufs=2, space="PSUM") as ps_sm,
            tc.tile_pool(name="ps_tp", bufs=2, space="PSUM") as ps_tp,
            tc.tile_pool(name="dram", bufs=1, space="DRAM") as dram,
        ):
            # ---- identities & constants ----
            # (gpsimd memset/affine_select reject f32r -> build f32, DVE-copy)
            identf = small.tile([128, 128], F32)
            make_identity(nc, identf)
            identr = small.tile([128, 128], F32R)
            nc.vector.tensor_copy(identr, identf)
            identb = small.tile([128, 128], BF16)
            make_identity(nc, identb)
            # f32r moving operands need even free size: ones2 = [1, 0] cols
            ones2f = small.tile([128, 2], F32)
            nc.vector.memset(ones2f, 0.0)
            nc.vector.memset(ones2f[:, 0:1], 1.0)
            ones2 = small.tile([128, 2], F32R)
            nc.vector.tensor_copy(ones2, ones2f)
            ones1b = small.tile([1, B], BF16)
            nc.vector.memset(ones1b, 1.0)

            # ---- small loads (head of the sync FIFO) ----
            with nc.named_scope("smalls"):
                pri16 = small.tile([IC, 128], F32)
                nc.sync.dma_start(out=pri16,
                                  in_=pri.ap().rearrange("(c p) -> c p", p=128))
                ages16 = small.tile([IC, 128], F32)
                nc.sync.dma_start(out=ages16,
                                  in_=ages.ap().rearrange("(c p) -> c p", p=128))
                val16 = small.tile([IC, 128], F32)
                nc.sync.dma_start(out=val16,
                                  in_=validf.ap().rearrange("(c p) -> c p", p=128))
                am_sb = small.tile([BL, T], F32R)
                nc.sync.dma_start(out=am_sb, in_=am[:, :])
                bq_sb = small.tile([1, JL], BF16)
                nc.gpsimd.dma_start(out=bq_sb,
                                    in_=bq.ap().rearrange("(a j) -> a j", a=1))
                bc_sb = small.tile([1, JL], BF16)
                nc.gpsimd.dma_start(out=bc_sb,
                                    in_=bc.ap().rearrange("(a j) -> a j", a=1))


            # ---- g vector (scores gate), 1/sqrt(D) folded in ----
            with nc.named_scope("gvec"):
                eff16 = small.tile([IC, 128], F32)
                nc.scalar.activation(eff16, ages16, AF.Exp, scale=math.log(DECAY))
                nc.vector.tensor_mul(eff16, eff16, pri16)
                negthr = small.tile([IC, 1], F32)
                nc.vector.memset(negthr, -10.0 * THR)
                g16 = small.tile([IC, 128], F32)
                nc.scalar.activation(g16, eff16, AF.Sigmoid, scale=10.0,
                                     bias=negthr[:, 0:1])
                nc.vector.tensor_mul(g16, g16, eff16)
                nc.vector.tensor_mul(g16, g16, val16)
                nc.vector.tensor_scalar_mul(g16, g16, 1.0 / math.sqrt(D))

                g_sb = small.tile([128, IC], F32)
                val_sb = small.tile([128, IC], F32)
                for src, dst in ((g16, g_sb), (val16, val_sb)):
                    ps = ps_tp.tile([128, 512], F32, tag="tp")
                    nc.tensor.transpose(ps[:, :IC], src, identf[:IC, :IC])
                    nc.vector.tensor_copy(dst, ps[:, :IC])

                # amT [128, TC, BL] f32r + row-sum reciprocal via PE
                amT = small.tile([128, TC, BL], F32R)
                aps = ps_tp.tile([128, 512], F32R, tag="tp")
                for tch in range(TC):
                    nc.tensor.transpose(aps[:, tch * BL:(tch + 1) * BL],
                                        am_sb[:, tch * 128:(tch + 1) * 128],
                                        identr[:BL, :BL])
                nc.vector.tensor_copy(amT, aps[:, :TC * BL])
                ms_ps = ps_sm.tile([B, 512], F32, tag="sm")
                for tch in range(TC):
                    nc.tensor.matmul(ms_ps[:BL, 0:2], amT[:, tch, :], ones2,
                                     start=(tch == 0), stop=(tch == TC - 1))
                ms8 = small.tile([BL, 1], F32)
                nc.vector.tensor_copy(ms8, ms_ps[:BL, 0:1])
                mt_ps = ps_tp.tile([128, 512], F32, tag="tp")
                nc.tensor.transpose(mt_ps[:1, :BL], ms8, identf[:BL, :BL])
                rmask = small.tile([1, BL], F32)
                nc.vector.tensor_scalar_add(rmask, mt_ps[:1, :BL], 1e-8)
                nc.vector.reciprocal(rmask, rmask)

            # ---- masked mean pooling; per-batch stores on the scalar ring ----
            # keep-warm matmuls (dead writes into the transpose PSUM pool)
            # hold the PE pstate up between data-gated bursts; the pstate /
            # HAM clock gate otherwise re-cools during each ~4us idle and
            # every burst pays the 2-3.7x cold-issue rate.
            def warm_mm(src_ap):
                wps = ps_tp.tile([128, 512], F32, tag="tp")
                nc.tensor.matmul(wps, identr, src_ap, start=True, stop=True)

            pooled_in = dram.tile([BL, D], BF16)
            with nc.named_scope("pool"):
                for b in range(BL):
                    pps = ps_acc.tile([1, D], F32, tag="acc")
                    for h in range(2):
                        qt_ = stream.tile([128, 2, D], F32R, tag="st")
                        nc.sync.dma_start(
                            out=qt_,
                            in_=qs[b, h * 256:(h + 1) * 256, :].rearrange(
                                "(tc p) d -> p tc d", p=128))
                        for t2 in range(2):
                            tch = h * 2 + t2
                            for nsl in range(4):
                                nc.tensor.matmul(
                                    pps[:, nsl * 512:(nsl + 1) * 512],
                                    amT[:, tch, b:b + 1],
                                    qt_[:, t2, nsl * 512:(nsl + 1) * 512],
                                    start=(tch == 0), stop=(tch == TC - 1),
                                )
                        warm_mm(qt_[:, 1, 0:512])
                    prow = stage.tile([1, D], BF16, tag="prow")
                    nc.vector.tensor_scalar_mul(prow, pps, rmask[:, b:b + 1])
                    nc.scalar.dma_start(out=pooled_in[b:b + 1, :], in_=prow)

            # ---- Wq load + transpose -> wqT bf16 [128, DC, JL] ----
            wqT = small.tile([128, DC, JL], BF16)
            with nc.named_scope("wqtr"):
                wtile = stream.tile([128, 2, D], F32R, tag="st")
                nc.sync.dma_start(
                    out=wtile, in_=wq.ap().rearrange("(jc p) d -> p jc d", p=128))
                for jc in range(2):
                    for dcg in range(4):
                        ps = ps_tp.tile([128, 512], F32R, tag="tp")
                        for d4 in range(4):
                            dc = dcg * 4 + d4
                            nc.tensor.transpose(
                                ps[:, d4 * 128:(d4 + 1) * 128],
                                wtile[:, jc, dc * 128:(dc + 1) * 128], identr)
                        nc.vector.tensor_copy(
                            wqT[:, dcg * 4:(dcg + 1) * 4,
                                jc * 128:(jc + 1) * 128],
                            ps.rearrange("p (d4 j) -> p d4 j", d4=4))

            # ---- AllGather #1 trigger (gpsimd; queue is otherwise idle) ----
            pooled_out = dram.tile([B, D], BF16)
            with nc.named_scope("ag1"):
                nc.gpsimd.collective_compute(
                    "AllGather", mybir.AluOpType.bypass, replica_groups=rg,
                    ins=[pooled_in.opt()], outs=[pooled_out.opt()],
                )

            # ---- K load + transpose -> kt bf16 [128, DC, NL] ----
            kt = persist.tile([128, DC, NL], BF16)
            pooled_full = small.tile([B, D], BF16)
            def emit_k_group(g):
                ktile = stream.tile([128, 2, D], F32R, tag="st")
                nc.sync.dma_start(
                    out=ktile,
                    in_=keys[g * 256:(g + 1) * 256, :].rearrange(
                        "(ic p) d -> p ic d", p=128))
                for sub in range(2):
                    ic = g * 2 + sub
                    for dcg in range(4):
                        ps = ps_tp.tile([128, 512], F32R, tag="tp")
                        for d4 in range(4):
                            dc = dcg * 4 + d4
                            nc.tensor.transpose(
                                ps[:, d4 * 128:(d4 + 1) * 128],
                                ktile[:, sub, dc * 128:(dc + 1) * 128],
                                identr)
                        nc.vector.tensor_copy(
                            kt[:, dcg * 4:(dcg + 1) * 4,
                               ic * 128:(ic + 1) * 128],
                            ps.rearrange("p (d4 j) -> p d4 j", d4=4))
            with nc.named_scope("ktr"):
                for g in range(5):
                    emit_k_group(g)
            # v6: AG#1-dependent gather load rides the idle SWDGE queue so the
            # sync FIFO (bulk K/V stream) never pauses for the collective
            nc.gpsimd.dma_start(out=pooled_full, in_=pooled_out[:, :])
            # ---- pooledT from the gathered pooled rows ----
            with nc.named_scope("ptr"):
                pooledT = small.tile([128, DC, B], BF16)
                for dcg in range(4):
                    ps = ps_tp.tile([128, 4, B], BF16, tag="tp")
                    for d4 in range(4):
                        dc = dcg * 4 + d4
                        nc.tensor.transpose(
                            ps[:, d4, :],
                            pooled_full[:, dc * 128:(dc + 1) * 128],
                            identb[:B, :B])
                    nc.vector.tensor_copy(pooledT[:, dcg * 4:(dcg + 1) * 4, :], ps)

            # ---- q slice = pooled @ WqT + bq, transposed, AllGather ----
            qt_in = dram.tile([JL, B], BF16)
            qt_out = dram.tile([D, B], BF16)
            with nc.named_scope("qproj"):
                qps = ps_sm.tile([B, 512], F32, tag="sm")
                for dc in range(DC):
                    nc.tensor.matmul(qps[:, :JL], pooledT[:, dc, :], wqT[:, dc, :],
                                     start=(dc == 0), stop=False)
                nc.tensor.matmul(qps[:, :JL], ones1b, bq_sb, start=False, stop=True)
                q_sb = small.tile([B, JL], F32R)
                nc.vector.tensor_copy(q_sb, qps[:, :JL])
                qt_ps = ps_tp.tile([128, 2, B], F32R, tag="tp")
                for jc in range(2):
                    nc.tensor.transpose(qt_ps[:, jc, :],
                                        q_sb[:, jc * 128:(jc + 1) * 128],
                                        identr[:B, :B])
                qT_slice = small.tile([128, 2, B], BF16)
                nc.vector.tensor_copy(qT_slice, qt_ps)
                nc.gpsimd.dma_start(
                    out=qt_in.rearrange("(c p) b -> p c b", p=128), in_=qT_slice)

            with nc.named_scope("ag2"):
                nc.gpsimd.collective_compute(
                    "AllGather", mybir.AluOpType.bypass, replica_groups=rg,
                    ins=[qt_in.opt()], outs=[qt_out.opt()],
                )
                # v6: AG#2-dependent loads on SWDGE too (keeps both HWDGE
                # FIFOs pause-free)
                qT = small.tile([128, DC, B], BF16)
                nc.gpsimd.dma_start(
                    out=qT,
                    in_=qt_out[:, :].rearrange("(c p) b -> p c b", p=128))

            with nc.named_scope("ktr_b"):
                for g in range(5, 8):
                    emit_k_group(g)

            # ---- V loads (consumed by EV below) ----
            vt_tiles = []
            with nc.named_scope("vload"):
                for g in range(8):
                    vt = stream.tile([128, 2, D], F32R, tag="st")
                    nc.sync.dma_start(
                        out=vt,
                        in_=values[g * 256:(g + 1) * 256, :].rearrange(
                            "(ic p) d -> p ic d", p=128))
                    vt_tiles.append(vt)


            # keep-warm matmuls paced by the first V-tile arrivals bridge the
            # PE idle window between the K transposes and the qT-gated scores
            # burst (vt0-2 land before qT comes back from AG#2, so these
            # cannot delay scores)
            for g in range(3):
                warm_mm(vt_tiles[g][:, 0, 0:512])

            # ---- scores -> exp -> et f32r [128, IC, B] ----
            et = small.tile([128, IC, B], F32R)
            with nc.named_scope("scores"):
                for nsl in range(4):
                    bps = ps_sm.tile([B, 512], F32, tag="sm")
                    for dc in range(DC):
                        nc.tensor.matmul(
                            bps, qT[:, dc, :], kt[:, dc, nsl * 512:(nsl + 1) * 512],
                            start=(dc == 0), stop=(dc == DC - 1),
                        )
                    bsl = stage.tile([B, 512], F32R, tag="bsl")
                    nc.vector.tensor_copy(bsl, bps)
                    tp = ps_tp.tile([128, 4, B], F32R, tag="tp")
                    for j in range(4):
                        nc.tensor.transpose(tp[:, j, :],
                                            bsl[:, j * 128:(j + 1) * 128],
                                            identr[:B, :B])
                    for j in range(4):
                        ic = nsl * 4 + j
                        nc.scalar.activation(et[:, ic, :], tp[:, j, :], AF.Exp,
                                             scale=g_sb[:, ic:ic + 1])
                        nc.vector.tensor_scalar_mul(et[:, ic, :], et[:, ic, :],
                                                    val_sb[:, ic:ic + 1])

            # ---- local row sums (PE ones-reduction) + early rowsT store ----
            ar_in = dram.tile([D + 1, B], BF16)
            ar_out = dram.tile([D + 1, B], BF16)
            with nc.named_scope("rows"):
                rs_ps = ps_sm.tile([B, 512], F32, tag="sm")
                for ic in range(IC):
                    nc.tensor.matmul(rs_ps[:, 0:2], et[:, ic, :], ones2,
                                     start=(ic == 0), stop=(ic == IC - 1))
                rows_sb = small.tile([B, 1], BF16)
                nc.vector.tensor_copy(rows_sb, rs_ps[:, 0:1])
                rt_ps = ps_tp.tile([128, 4, B], BF16, tag="tp")
                nc.tensor.transpose(rt_ps[:1, 0, :], rows_sb, identb[:B, :B])
                rowsT = small.tile([1, B], BF16)
                nc.vector.tensor_copy(rowsT, rt_ps[:1, 0, :])
                nc.gpsimd.dma_start(out=ar_in[D:D + 1, :], in_=rowsT)

            # ---- partial attention output E @ V (unnormalized) ----
            with nc.named_scope("ev"):
                ev_ps = ps_acc.tile([B, D], F32, tag="acc")
                for g in range(7):
                    vt = vt_tiles[g]
                    for sub in range(2):
                        ic = g * 2 + sub
                        for nsl in range(4):
                            nc.tensor.matmul(
                                ev_ps[:, nsl * 512:(nsl + 1) * 512],
                                et[:, ic, :], vt[:, sub, nsl * 512:(nsl + 1) * 512],
                                start=(ic == 0), stop=False,
                            )
                # last V tile with nsl OUTER: each 512-col slice (= one PSUM
                # bank) finalizes as early as possible, so its bf16 copy +
                # transpose + ar_in store overlap the remaining slices' mms
                # instead of forming a ~6us serial tail before the AllReduce
                vt = vt_tiles[7]
                attnT = small.tile([128, DC, B], BF16)
                for nsl in range(4):
                    for sub in range(2):
                        nc.tensor.matmul(
                            ev_ps[:, nsl * 512:(nsl + 1) * 512],
                            et[:, 14 + sub, :],
                            vt[:, sub, nsl * 512:(nsl + 1) * 512],
                            start=False, stop=(sub == 1),
                        )
                    asl = stage.tile([B, 512], BF16, tag="bsl")
                    nc.vector.tensor_copy(
                        asl, ev_ps[:, nsl * 512:(nsl + 1) * 512])
                    ps = ps_tp.tile([128, 4, B], BF16, tag="tp")
                    for d4 in range(4):
                        nc.tensor.transpose(ps[:, d4, :],
                                            asl[:, d4 * 128:(d4 + 1) * 128],
                                            identb[:B, :B])
                    nc.vector.tensor_copy(attnT[:, nsl * 4:(nsl + 1) * 4, :], ps)
                    nc.gpsimd.dma_start(
                        out=ar_in[nsl * 512:(nsl + 1) * 512, :].rearrange(
                            "(dc p) b -> p dc b", p=128),
                        in_=attnT[:, nsl * 4:(nsl + 1) * 4, :])

            # ---- Wc load + transpose -> wcT bf16 ----
            # (v7: streamed AFTER V — wcT is only needed post-AllReduce, so
            # this takes Wc's 5.6us off the EV-completion critical path)
            wcT = small.tile([128, DC, JL], BF16)
            with nc.named_scope("wctr"):
                wtile = stream.tile([128, 2, D], F32R, tag="st")
                nc.sync.dma_start(
                    out=wtile, in_=wc.ap().rearrange("(jc p) d -> p jc d", p=128))
                for jc in range(2):
                    for dcg in range(4):
                        ps = ps_tp.tile([128, 512], F32R, tag="tp")
                        for d4 in range(4):
                            dc = dcg * 4 + d4
                            nc.tensor.transpose(
                                ps[:, d4 * 128:(d4 + 1) * 128],
                                wtile[:, jc, dc * 128:(dc + 1) * 128], identr)
                        nc.vector.tensor_copy(
                            wcT[:, dcg * 4:(dcg + 1) * 4,
                                jc * 128:(jc + 1) * 128],
                            ps.rearrange("p (d4 j) -> p d4 j", d4=4))

            # ---- fused AllReduce of [partial E@V || row sums]  [D+1, B] ----
            with nc.named_scope("ar"):
                nc.gpsimd.collective_compute(
                    "AllReduce", mybir.AluOpType.add, replica_groups=rg,
                    ins=[ar_in.opt()], outs=[ar_out.opt()],
                )
                denrow = small.tile([1, B], BF16)
                nc.gpsimd.dma_start(out=denrow, in_=ar_out[D:D + 1, :])
                dr_ps = ps_tp.tile([128, 4, B], BF16, tag="tp")
                nc.tensor.transpose(dr_ps[:B, 0, 0:1], denrow, identb[:1, :1])
                denom = small.tile([B, 1], F32)
                nc.vector.tensor_copy(denom, dr_ps[:B, 0, 0:1])
                rinv = small.tile([B, 1], F32)
                nc.vector.reciprocal(rinv, denom)

            # ---- final = (out_full @ WcT + denom*bc) / denom ----
            with nc.named_scope("fin"):
                outT = small.tile([128, DC, B], BF16)
                nc.gpsimd.dma_start(
                    out=outT,
                    in_=ar_out[0:D, :].rearrange("(dc p) b -> p dc b", p=128))
                fin_ps = ps_sm.tile([B, 512], F32, tag="sm")
                for dc in range(DC):
                    nc.tensor.matmul(fin_ps[:, :JL], outT[:, dc, :], wcT[:, dc, :],
                                     start=(dc == 0), stop=False)
                nc.tensor.matmul(fin_ps[:, :JL], denrow, bc_sb,
                                 start=False, stop=True)
                fin_sb = small.tile([B, JL], F32)
                nc.vector.tensor_scalar_mul(fin_sb, fin_ps[:, :JL], rinv[:, 0:1])
                nc.sync.dma_start(out=y[:, :], in_=fin_sb)

    nc.compile()
    return nc


def get_nc():
    global _NC_CACHE
    if _NC_CACHE is None:
        _NC_CACHE = build_nc()
    return _NC_CACHE


def make_in_maps(inputs):
    qs = np.ascontiguousarray(np.asarray(inputs["query_states"], np.float32))
    am = np.ascontiguousarray(np.asarray(inputs["attention_mask"], np.float32))
    keys = np.ascontiguousarray(np.asarray(inputs["keys"], np.float32))
    values = np.ascontiguousarray(np.asarray(inputs["values"], np.float32))
    pri = np.ascontiguousarray(np.asarray(inputs["priorities"], np.float32))
    ages = np.ascontiguousarray(np.asarray(inputs["ages"], np.float32))
    validf = np.ascontiguousarray(np.asarray(inputs["valid_mask"]).astype(np.float32))
    Wq = np.ascontiguousarray(np.asarray(inputs["Wq"], np.float32))
    bq = np.ascontiguousarray(np.asarray(inputs["bq"], np.float32))
    Wc = np.ascontiguousarray(np.asarray(inputs["Wc"], np.float32))
    bc = np.ascontiguousarray(np.asarray(inputs["bc"], np.float32))

    in_maps = []
    for c in range(N_CORES):
        in_maps.append({
            "qs": qs[c * BL:(c + 1) * BL],
            "am": am[c * BL:(c + 1) * BL],
            "keys": keys[c * NL:(c + 1) * NL],
            "values": values[c * NL:(c + 1) * NL],
            "pri": pri[c * NL:(c + 1) * NL],
            "ages": ages[c * NL:(c + 1) * NL],
            "validf": validf[c * NL:(c + 1) * NL],
            "wq": Wq[c * JL:(c + 1) * JL],
            "bq": bq[c * JL:(c + 1) * JL],
            "wc": Wc[c * JL:(c + 1) * JL],
            "bc": bc[c * JL:(c + 1) * JL],
        })
    return in_maps


def kernel(**inputs) -> np.ndarray:
    nc = get_nc()
    res = run_bass_kernel_spmd(nc, make_in_maps(inputs),
                               core_ids=list(range(N_CORES)))
    return np.concatenate([res.results[c]["out"] for c in range(N_CORES)], axis=1)


if __name__ == "__main__":
    build_nc()
    print("kernel built OK")



# revision 43
# speedup vs baseline: 1.7192x; 1.7192x over previous
"""Trainium2 Bass kernel for nn_DifferentiablePriorityBuffer (8 NeuronCores), v2.

Same math as v1 (single-round closed form, rel err ~9e-7 vs 10-round scan):

    pooled = (qs * mask).sum(T) / (mask.sum(T) + 1e-8)          (B, D)
    q      = (pooled @ Wq.T + bq)                               (B, D)
    g      = sigmoid((eff - 0.5) * 10) * eff * valid / sqrt(D)
    E      = exp((q @ K.T) * g) * valid                         (B, N)
    out    = (E @ V) / E.sum(-1, keepdims=True)                 (B, D)
    final  = out @ Wc.T + bc                                    (B, D)

v7 (this file) vs v5:
  - Pause-free bulk stream: the AG#1/AG#2-dependent small loads (pooled_full,
    qT) ride the idle SWDGE (gpsimd) queue instead of pausing the HWDGE sync
    FIFO, so the 68MB/core qs+Wq+K+V+Wc stream never yields to a collective.
  - K groups emitted contiguously; the pooledT/qproj/AG#2 block sits between
    K groups 4/5 so its PE work overlaps the K stream without blocking the
    later K transposes for long.
  - Wc streamed AFTER V (wcT is only needed post-AllReduce), taking its
    5.6us off the EV-completion critical path.
  - stream pool 5 bufs -> 10MB of V prefetched across the scores/et wait.
  - Keep-warm matmuls in the pooling phase and across the K-transpose ->
    scores idle window (paced by V-tile arrivals) hold the PE pstate (HAM
    clock gate) up between data-gated bursts.
  - qT / outT gathers merged into single SWDGE DMAs.
  - Last V tile processed nsl-outer: each 512-col E@V slice finalizes as
    soon as possible and its transpose + ar_in store overlaps the other
    slices' matmuls, cutting the EV -> AllReduce serial tail from ~6us
    to ~2us.

v5 scheduling changes vs v1 (v1: 319us NEFF, DMA busy 200us with a 110us
compute/collective tail):
  - All bulk loads (qs, K, V, Wq, Wc) moved to HWDGE (nc.sync) in f32:
    the sync FIFO serializes them in priority order qs > Wq > K > Wc > V
    while streaming at full HBM rate, and the GpSimd queue only carries
    collectives + small transfers, so collective triggers fire as soon
    as their data is ready (v1 lost ~34us to K-load DMAs queued ahead of
    the qT AllGather trigger).
  - float32r matmuls consume the f32 data directly (1 cycle/row at
    >=256 moving cols, vs 4 for plain f32) - no bf16 cast DMAs needed.
    K/W are cast to bf16 on the PSUM->SBUF copies after their PE
    transposes.
  - A dummy warmup AllGather absorbs the one-time ~11us collective
    trigger-start delay while DMA streams.
  - All PSUM->SBUF copies on the vector engine (scalar ACT copies are
    2-9x slower); scalar only runs activations.
  - One shared streaming pool (qs/K/V/W tiles, 5 x 2MB) keeps SBUF
    bounded and lets buffer recycling pace the load FIFO.
"""
import sys

if "/opt/trn_rl_repo" not in sys.path:
    sys.path.insert(0, "/opt/trn_rl_repo")

import math

import numpy as np

import concourse.bacc as bacc
import concourse.tile as tile
from concourse import mybir
from concourse.bass_utils import run_bass_kernel_spmd
from concourse.masks import make_identity

N_CORES = 8
B, T, D = 64, 512, 2048
N = 16384
DECAY = 0.9
THR = 0.5
BL = B // N_CORES          # 8 batches per core
NL = N // N_CORES          # 2048 buffer rows per core
JL = D // N_CORES          # 256 output features per core
DC = D // 128              # 16 contraction chunks
IC = NL // 128             # 16 local buffer chunks
TC = T // 128              # 4 time chunks
F32 = mybir.dt.float32
F32R = mybir.dt.float32r
BF16 = mybir.dt.bfloat16
AF = mybir.ActivationFunctionType

_NC_CACHE = None


def build_nc():
    nc = bacc.Bacc("TRN2", target_bir_lowering=False, debug=False,
                   num_devices=N_CORES)

    qs = nc.dram_tensor("qs", [BL, T, D], F32R, kind="ExternalInput")
    am = nc.dram_tensor("am", [BL, T], F32R, kind="ExternalInput")
    keys = nc.dram_tensor("keys", [NL, D], F32R, kind="ExternalInput")
    values = nc.dram_tensor("values", [NL, D], F32R, kind="ExternalInput")
    pri = nc.dram_tensor("pri", [NL], F32, kind="ExternalInput")
    ages = nc.dram_tensor("ages", [NL], F32, kind="ExternalInput")
    validf = nc.dram_tensor("validf", [NL], F32, kind="ExternalInput")
    wq = nc.dram_tensor("wq", [JL, D], F32R, kind="ExternalInput")
    bq = nc.dram_tensor("bq", [JL], F32, kind="ExternalInput")
    wc = nc.dram_tensor("wc", [JL, D], F32R, kind="ExternalInput")
    bc = nc.dram_tensor("bc", [JL], F32, kind="ExternalInput")
    y = nc.dram_tensor("out", [B, JL], F32, kind="ExternalOutput")

    rg = [list(range(N_CORES))]

    with tile.TileContext(nc) as tc:
        with (
            tc.tile_pool(name="small", bufs=1) as small,
            tc.tile_pool(name="stream", bufs=5) as stream,
            tc.tile_pool(name="persist", bufs=1) as persist,
            tc.tile_pool(name="stage", bufs=2) as stage,
            tc.tile_pool(name="ps_acc", bufs=1, space="PSUM") as ps_acc,
            tc.tile_pool(name="ps_sm", bufs=2, space="PSUM") as ps_sm,
            tc.tile_pool(name="ps_tp", bufs=2, space="PSUM") as ps_tp,
            tc.tile_pool(name="dram", bufs=1, space="DRAM") as dram,
        ):
            # ---- identities & constants ----
            # (gpsimd memset/affine_select reject f32r -> build f32, DVE-copy)
            identf = small.tile([128, 128], F32)
            make_identity(nc, identf)
            identr = small.tile([128, 128], F32R)
            nc.vector.tensor_copy(identr, identf)
            identb = small.tile([128, 128], BF16)
            make_identity(nc, identb)
            # f32r moving operands need even free size: ones2 = [1, 0] cols
            ones2f = small.tile([128, 2], F32)
            nc.vector.memset(ones2f, 0.0)
            nc.vector.memset(ones2f[:, 0:1], 1.0)
            ones2 = small.tile([128, 2], F32R)
            nc.vector.tensor_copy(ones2, ones2f)
            ones1b = small.tile([1, B], BF16)
            nc.vector.memset(ones1b, 1.0)

            # ---- small loads (head of the sync FIFO) ----
            with nc.named_scope("smalls"):
                pri16 = small.tile([IC, 128], F32)
                nc.sync.dma_start(out=pri16,
                                  in_=pri.ap().rearrange("(c p) -> c p", p=128))
                ages16 = small.tile([IC, 128], F32)
                nc.sync.dma_start(out=ages16,
                                  in_=ages.ap().rearrange("(c p) -> c p", p=128))
                val16 = small.tile([IC, 128], F32)
                nc.sync.dma_start(out=val16,
                                  in_=validf.ap().rearrange("(c p) -> c p", p=128))
                am_sb = small.tile([BL, T], F32R)
                nc.sync.dma_start(out=am_sb, in_=am[:, :])
                bq_sb = small.tile([1, JL], BF16)
                nc.gpsimd.dma_start(out=bq_sb,
                                    in_=bq.ap().rearrange("(a j) -> a j", a=1))
                bc_sb = small.tile([1, JL], BF16)
                nc.gpsimd.dma_start(out=bc_sb,
                                    in_=bc.ap().rearrange("(a j) -> a j", a=1))


            # ---- g vector (scores gate), 1/sqrt(D) folded in ----
            with nc.named_scope("gvec"):
                eff16 = small.tile([IC, 128], F32)
                nc.scalar.activation(eff16, ages16, AF.Exp, scale=math.log(DECAY))
                nc.vector.tensor_mul(eff16, eff16, pri16)
                negthr = small.tile([IC, 1], F32)
                nc.vector.memset(negthr, -10.0 * THR)
                g16 = small.tile([IC, 128], F32)
                nc.scalar.activation(g16, eff16, AF.Sigmoid, scale=10.0,
                                     bias=negthr[:, 0:1])
                nc.vector.tensor_mul(g16, g16, eff16)
                nc.vector.tensor_mul(g16, g16, val16)
                nc.vector.tensor_scalar_mul(g16, g16, 1.0 / math.sqrt(D))

                g_sb = small.tile([128, IC], F32)
                val_sb = small.tile([128, IC], F32)
                for src, dst in ((g16, g_sb), (val16, val_sb)):
                    ps = ps_tp.tile([128, 512], F32, tag="tp")
                    nc.tensor.transpose(ps[:, :IC], src, identf[:IC, :IC])
                    nc.vector.tensor_copy(dst, ps[:, :IC])

                # amT [128, TC, BL] f32r + row-sum reciprocal via PE
                amT = small.tile([128, TC, BL], F32R)
                aps = ps_tp.tile([128, 512], F32R, tag="tp")
                for tch in range(TC):
                    nc.tensor.transpose(aps[:, tch * BL:(tch + 1) * BL],
                                        am_sb[:, tch * 128:(tch + 1) * 128],
                                        identr[:BL, :BL])
                nc.vector.tensor_copy(amT, aps[:, :TC * BL])
                ms_ps = ps_sm.tile([B, 512], F32, tag="sm")
                for tch in range(TC):
                    nc.tensor.matmul(ms_ps[:BL, 0:2], amT[:, tch, :], ones2,
                                     start=(tch == 0), stop=(tch == TC - 1))
                ms8 = small.tile([BL, 1], F32)
                nc.vector.tensor_copy(ms8, ms_ps[:BL, 0:1])
                mt_ps = ps_tp.tile([128, 512], F32, tag="tp")
                nc.tensor.transpose(mt_ps[:1, :BL], ms8, identf[:BL, :BL])
                rmask = small.tile([1, BL], F32)
                nc.vector.tensor_scalar_add(rmask, mt_ps[:1, :BL], 1e-8)
                nc.vector.reciprocal(rmask, rmask)

            # ---- masked mean pooling; per-batch stores on the scalar ring ----
            # keep-warm matmuls (dead writes into the transpose PSUM pool)
            # hold the PE pstate up between data-gated bursts; the pstate /
            # HAM clock gate otherwise re-cools during each ~4us idle and
            # every burst pays the 2-3.7x cold-issue rate.
            def warm_mm(src_ap):
                wps = ps_tp.tile([128, 512], F32, tag="tp")
                nc.tensor.matmul(wps, identr, src_ap, start=True, stop=True)

            pooled_in = dram.tile([BL, D], BF16)
            with nc.named_scope("pool"):
                for b in range(BL):
                    pps = ps_acc.tile([1, D], F32, tag="acc")
                    for h in range(2):
                        qt_ = stream.tile([128, 2, D], F32R, tag="st")
                        nc.sync.dma_start(
                            out=qt_,
                            in_=qs[b, h * 256:(h + 1) * 256, :].rearrange(
                                "(tc p) d -> p tc d", p=128))
                        for t2 in range(2):
                            tch = h * 2 + t2
                            for nsl in range(4):
                                nc.tensor.matmul(
                                    pps[:, nsl * 512:(nsl + 1) * 512],
                                    amT[:, tch, b:b + 1],
                                    qt_[:, t2, nsl * 512:(nsl + 1) * 512],
                                    start=(tch == 0), stop=(tch == TC - 1),
                                )
                        warm_mm(qt_[:, 1, 0:512])
                    prow = stage.tile([1, D], BF16, tag="prow")
                    nc.vector.tensor_scalar_mul(prow, pps, rmask[:, b:b + 1])
                    nc.scalar.dma_start(out=pooled_in[b:b + 1, :], in_=prow)

            # ---- Wq load + transpose -> wqT bf16 [128, DC, JL] ----
            wqT = small.tile([128, DC, JL], BF16)
            with nc.named_scope("wqtr"):
                wtile = stream.tile([128, 2, D], F32R, tag="st")
                nc.sync.dma_start(
                    out=wtile, in_=wq.ap().rearrange("(jc p) d -> p jc d", p=128))
                for jc in range(2):
                    for dcg in range(4):
                        ps = ps_tp.tile([128, 512], F32R, tag="tp")
                        for d4 in range(4):
                            dc = dcg * 4 + d4
                            nc.tensor.transpose(
                                ps[:, d4 * 128:(d4 + 1) * 128],
                                wtile[:, jc, dc * 128:(dc + 1) * 128], identr)
                        nc.vector.tensor_copy(
                            wqT[:, dcg * 4:(dcg + 1) * 4,
                                jc * 128:(jc + 1) * 128],
                            ps.rearrange("p (d4 j) -> p d4 j", d4=4))

            # ---- AllGather #1 trigger (gpsimd; queue is otherwise idle) ----
            pooled_out = dram.tile([B, D], BF16)
            with nc.named_scope("ag1"):
                nc.gpsimd.collective_compute(
                    "AllGather", mybir.AluOpType.bypass, replica_groups=rg,
                    ins=[pooled_in.opt()], outs=[pooled_out.opt()],
                )

            # ---- K load + transpose -> kt bf16 [128, DC, NL] ----
            kt = persist.tile([128, DC, NL], BF16)
            pooled_full = small.tile([B, D], BF16)
            def emit_k_group(g):
                ktile = stream.tile([128, 2, D], F32R, tag="st")
                nc.sync.dma_start(
                    out=ktile,
                    in_=keys[g * 256:(g + 1) * 256, :].rearrange(
                        "(ic p) d -> p ic d", p=128))
                for sub in range(2):
                    ic = g * 2 + sub
                    for dcg in range(4):
                        ps = ps_tp.tile([128, 512], F32R, tag="tp")
                        for d4 in range(4):
                            dc = dcg * 4 + d4
                            nc.tensor.transpose(
                                ps[:, d4 * 128:(d4 + 1) * 128],
                                ktile[:, sub, dc * 128:(dc + 1) * 128],
                                identr)
                        nc.vector.tensor_copy(
                            kt[:, dcg * 4:(dcg + 1) * 4,
                               ic * 128:(ic + 1) * 128],
                            ps.rearrange("p (d4 j) -> p d4 j", d4=4))
            with nc.named_scope("ktr"):
                for g in range(5):
                    emit_k_group(g)
            # v6: AG#1-dependent gather load rides the idle SWDGE queue so the
            # sync FIFO (bulk K/V stream) never pauses for the collective
            nc.gpsimd.dma_start(out=pooled_full, in_=pooled_out[:, :])
            # ---- pooledT from the gathered pooled rows ----
            with nc.named_scope("ptr"):
                pooledT = small.tile([128, DC, B], BF16)
                for dcg in range(4):
                    ps = ps_tp.tile([128, 4, B], BF16, tag="tp")
                    for d4 in range(4):
                        dc = dcg * 4 + d4
                        nc.tensor.transpose(
                            ps[:, d4, :],
                            pooled_full[:, dc * 128:(dc + 1) * 128],
                            identb[:B, :B])
                    nc.vector.tensor_copy(pooledT[:, dcg * 4:(dcg + 1) * 4, :], ps)

            # ---- q slice = pooled @ WqT + bq, transposed, AllGather ----
            qt_in = dram.tile([JL, B], BF16)
            qt_out = dram.tile([D, B], BF16)
            with nc.named_scope("qproj"):
                qps = ps_sm.tile([B, 512], F32, tag="sm")
                for dc in range(DC):
                    nc.tensor.matmul(qps[:, :JL], pooledT[:, dc, :], wqT[:, dc, :],
                                     start=(dc == 0), stop=False)
                nc.tensor.matmul(qps[:, :JL], ones1b, bq_sb, start=False, stop=True)
                q_sb = small.tile([B, JL], F32R)
                nc.vector.tensor_copy(q_sb, qps[:, :JL])
                qt_ps = ps_tp.tile([128, 2, B], F32R, tag="tp")
                for jc in range(2):
                    nc.tensor.transpose(qt_ps[:, jc, :],
                                        q_sb[:, jc * 128:(jc + 1) * 128],
                                        identr[:B, :B])
                qT_slice = small.tile([128, 2, B], BF16)
                nc.vector.tensor_copy(qT_slice, qt_ps)
                nc.gpsimd.dma_start(
                    out=qt_in.rearrange("(c p) b -> p c b", p=128), in_=qT_slice)

            with nc.named_scope("ag2"):
                nc.gpsimd.collective_compute(
                    "AllGather", mybir.AluOpType.bypass, replica_groups=rg,
                    ins=[qt_in.opt()], outs=[qt_out.opt()],
                )
                # v6: AG#2-dependent loads on SWDGE too (keeps both HWDGE
                # FIFOs pause-free)
                qT = small.tile([128, DC, B], BF16)
                nc.gpsimd.dma_start(
                    out=qT,
                    in_=qt_out[:, :].rearrange("(c p) b -> p c b", p=128))

            with nc.named_scope("ktr_b"):
                for g in range(5, 8):
                    emit_k_group(g)

            # ---- V loads (consumed by EV below) ----
            vt_tiles = []
            with nc.named_scope("vload"):
                for g in range(8):
                    vt = stream.tile([128, 2, D], F32R, tag="st")
                    nc.sync.dma_start(
                        out=vt,
                        in_=values[g * 256:(g + 1) * 256, :].rearrange(
                            "(ic p) d -> p ic d", p=128))
                    vt_tiles.append(vt)


            # keep-warm matmuls paced by the first V-tile arrivals bridge the
            # PE idle window between the K transposes and the qT-gated scores
            # burst (vt0-2 land before qT comes back from AG#2, so these
            # cannot delay scores)
            for g in range(3):
                warm_mm(vt_tiles[g][:, 0, 0:512])


            # ---- scores nsl-blocks interleaved with EV groups ----
            # EV group g only needs et columns from scores block g//2, so each
            # pair of EV groups is emitted right after its producing block -
            # EV starts ~3us after scores nsl0 instead of ~14us after.
            ar_in = dram.tile([D + 1, B], BF16)
            ar_out = dram.tile([D + 1, B], BF16)
            et = small.tile([128, IC, B], F32R)
            ev_ps = ps_acc.tile([B, D], F32, tag="acc")

            def emit_scores_block(nsl):
                bps = ps_sm.tile([B, 512], F32, tag="sm")
                for dc in range(DC):
                    nc.tensor.matmul(
                        bps, qT[:, dc, :], kt[:, dc, nsl * 512:(nsl + 1) * 512],
                        start=(dc == 0), stop=(dc == DC - 1),
                    )
                bsl = stage.tile([B, 512], F32R, tag="bsl")
                nc.vector.tensor_copy(bsl, bps)
                tp = ps_tp.tile([128, 4, B], F32R, tag="tp")
                for j in range(4):
                    nc.tensor.transpose(tp[:, j, :],
                                        bsl[:, j * 128:(j + 1) * 128],
                                        identr[:B, :B])
                for j in range(4):
                    ic = nsl * 4 + j
                    nc.scalar.activation(et[:, ic, :], tp[:, j, :], AF.Exp,
                                         scale=g_sb[:, ic:ic + 1])
                    nc.vector.tensor_scalar_mul(et[:, ic, :], et[:, ic, :],
                                                val_sb[:, ic:ic + 1])

            def emit_ev_group(g):
                vt = vt_tiles[g]
                for sub in range(2):
                    ic = g * 2 + sub
                    for nsl in range(4):
                        nc.tensor.matmul(
                            ev_ps[:, nsl * 512:(nsl + 1) * 512],
                            et[:, ic, :], vt[:, sub, nsl * 512:(nsl + 1) * 512],
                            start=(ic == 0), stop=False,
                        )

            with nc.named_scope("scores_ev"):
                for nsl in range(4):
                    emit_scores_block(nsl)
                    for g in (2 * nsl, 2 * nsl + 1):
                        if g < 7:
                            emit_ev_group(g)

            # ---- local row sums (PE ones-reduction) + rowsT store ----
            with nc.named_scope("rows"):
                rs_ps = ps_sm.tile([B, 512], F32, tag="sm")
                for ic in range(IC):
                    nc.tensor.matmul(rs_ps[:, 0:2], et[:, ic, :], ones2,
                                     start=(ic == 0), stop=(ic == IC - 1))
                rows_sb = small.tile([B, 1], BF16)
                nc.vector.tensor_copy(rows_sb, rs_ps[:, 0:1])
                rt_ps = ps_tp.tile([128, 4, B], BF16, tag="tp")
                nc.tensor.transpose(rt_ps[:1, 0, :], rows_sb, identb[:B, :B])
                rowsT = small.tile([1, B], BF16)
                nc.vector.tensor_copy(rowsT, rt_ps[:1, 0, :])
                nc.gpsimd.dma_start(out=ar_in[D:D + 1, :], in_=rowsT)

            # ---- E @ V tail (last V tile, nsl-outer) ----
            with nc.named_scope("ev"):
                # last V tile with nsl OUTER: each 512-col slice (= one PSUM
                # bank) finalizes as early as possible, so its bf16 copy +
                # transpose + ar_in store overlap the remaining slices' mms
                # instead of forming a ~6us serial tail before the AllReduce
                vt = vt_tiles[7]
                attnT = small.tile([128, DC, B], BF16)
                for nsl in range(4):
                    for sub in range(2):
                        nc.tensor.matmul(
                            ev_ps[:, nsl * 512:(nsl + 1) * 512],
                            et[:, 14 + sub, :],
                            vt[:, sub, nsl * 512:(nsl + 1) * 512],
                            start=False, stop=(sub == 1),
                        )
                    asl = stage.tile([B, 512], BF16, tag="bsl")
                    nc.vector.tensor_copy(
                        asl, ev_ps[:, nsl * 512:(nsl + 1) * 512])
                    ps = ps_tp.tile([128, 4, B], BF16, tag="tp")
                    for d4 in range(4):
                        nc.tensor.transpose(ps[:, d4, :],
                                            asl[:, d4 * 128:(d4 + 1) * 128],
                                            identb[:B, :B])
                    nc.vector.tensor_copy(attnT[:, nsl * 4:(nsl + 1) * 4, :], ps)
                    nc.gpsimd.dma_start(
                        out=ar_in[nsl * 512:(nsl + 1) * 512, :].rearrange(
                            "(dc p) b -> p dc b", p=128),
                        in_=attnT[:, nsl * 4:(nsl + 1) * 4, :])

            # ---- Wc load + transpose -> wcT bf16 ----
            # (v7: streamed AFTER V — wcT is only needed post-AllReduce, so
            # this takes Wc's 5.6us off the EV-completion critical path)
            wcT = small.tile([128, DC, JL], BF16)
            with nc.named_scope("wctr"):
                wtile = stream.tile([128, 2, D], F32R, tag="st")
                nc.sync.dma_start(
                    out=wtile, in_=wc.ap().rearrange("(jc p) d -> p jc d", p=128))
                for jc in range(2):
                    for dcg in range(4):
                        ps = ps_tp.tile([128, 512], F32R, tag="tp")
                        for d4 in range(4):
                            dc = dcg * 4 + d4
                            nc.tensor.transpose(
                                ps[:, d4 * 128:(d4 + 1) * 128],
                                wtile[:, jc, dc * 128:(dc + 1) * 128], identr)
                        nc.vector.tensor_copy(
                            wcT[:, dcg * 4:(dcg + 1) * 4,
                                jc * 128:(jc + 1) * 128],
                            ps.rearrange("p (d4 j) -> p d4 j", d4=4))

            # ---- fused AllReduce of [partial E@V || row sums]  [D+1, B] ----
            with nc.named_scope("ar"):
                nc.gpsimd.collective_compute(
                    "AllReduce", mybir.AluOpType.add, replica_groups=rg,
                    ins=[ar_in.opt()], outs=[ar_out.opt()],
                )
                denrow = small.tile([1, B], BF16)
                nc.gpsimd.dma_start(out=denrow, in_=ar_out[D:D + 1, :])
                dr_ps = ps_tp.tile([128, 4, B], BF16, tag="tp")
                nc.tensor.transpose(dr_ps[:B, 0, 0:1], denrow, identb[:1, :1])
                denom = small.tile([B, 1], F32)
                nc.vector.tensor_copy(denom, dr_ps[:B, 0, 0:1])
                rinv = small.tile([B, 1], F32)
                nc.vector.reciprocal(rinv, denom)

            # ---- final = (out_full @ WcT + denom*bc) / denom ----
            with nc.named_scope("fin"):
                outT = small.tile([128, DC, B], BF16)
                nc.gpsimd.dma_start(
                    out=outT,
                    in_=ar_out[0:D, :].rearrange("(dc p) b -> p dc b", p=128))
                fin_ps = ps_sm.tile([B, 512], F32, tag="sm")
                for dc in range(DC):
                    nc.tensor.matmul(fin_ps[:, :JL], outT[:, dc, :], wcT[:, dc, :],
                                     start=(dc == 0), stop=False)
                nc.tensor.matmul(fin_ps[:, :JL], denrow, bc_sb,
                                 start=False, stop=True)
                fin_sb = small.tile([B, JL], F32)
                nc.vector.tensor_scalar_mul(fin_sb, fin_ps[:, :JL], rinv[:, 0:1])
                nc.sync.dma_start(out=y[:, :], in_=fin_sb)

    nc.compile()
    return nc


def get_nc():
    global _NC_CACHE
    if _NC_CACHE is None:
        _NC_CACHE = build_nc()
    return _NC_CACHE


def make_in_maps(inputs):
    qs = np.ascontiguousarray(np.asarray(inputs["query_states"], np.float32))
    am = np.ascontiguousarray(np.asarray(inputs["attention_mask"], np.float32))
    keys = np.ascontiguousarray(np.asarray(inputs["keys"], np.float32))
    values = np.ascontiguousarray(np.asarray(inputs["values"], np.float32))
    pri = np.ascontiguousarray(np.asarray(inputs["priorities"], np.float32))
    ages = np.ascontiguousarray(np.asarray(inputs["ages"], np.float32))
    validf = np.ascontiguousarray(np.asarray(inputs["valid_mask"]).astype(np.float32))
    Wq = np.ascontiguousarray(np.asarray(inputs["Wq"], np.float32))
    bq = np.ascontiguousarray(np.asarray(inputs["bq"], np.float32))
    Wc = np.ascontiguousarray(np.asarray(inputs["Wc"], np.float32))
    bc = np.ascontiguousarray(np.asarray(inputs["bc"], np.float32))

    in_maps = []
    for c in range(N_CORES):
        in_maps.append({
            "qs": qs[c * BL:(c + 1) * BL],
            "am": am[c * BL:(c + 1) * BL],
            "keys": keys[c * NL:(c + 1) * NL],
            "values": values[c * NL:(c + 1) * NL],
            "pri": pri[c * NL:(c + 1) * NL],
            "ages": ages[c * NL:(c + 1) * NL],
            "validf": validf[c * NL:(c + 1) * NL],
            "wq": Wq[c * JL:(c + 1) * JL],
            "bq": bq[c * JL:(c + 1) * JL],
            "wc": Wc[c * JL:(c + 1) * JL],
            "bc": bc[c * JL:(c + 1) * JL],
        })
    return in_maps


def kernel(**inputs) -> np.ndarray:
    nc = get_nc()
    res = run_bass_kernel_spmd(nc, make_in_maps(inputs),
                               core_ids=list(range(N_CORES)))
    return np.concatenate([res.results[c]["out"] for c in range(N_CORES)], axis=1)


if __name__ == "__main__":
    build_nc()
    print("kernel built OK")

